# revision 58
# baseline (speedup 1.0000x reference)
"""Absorbed-MLA Bass kernel for 8 trn2 cores, fp8/bf16 mixed precision.

Sharding: DP=2 over batch x TP=4 over heads (2 heads/core).
core c -> batch b=c//4, head-group g=c%4 (global heads 2g, 2g+1).

MLA absorption: scores_h = (c_q @ M_h + m0_h)*c_kv + q_rot*k_rot with
M_h = W_uq_h @ W_uk_h^T.  out = sum_h softmax_h @ c_kv @ U_h + b_o'.

Precision scheme (tolerance 2e-2 rel-to-max; HW measures ~4.9e-3):
  - down-proj: fp8 DoubleRow, 3 passes h8@W8 + dh8@W8 + h8@dW8 where
    dX8 = fp8(X - fp8(X)) is an unscaled fp8 residual -- bf16-grade
    accuracy at ~0.75x the bf16 PE cost, contracting 256 dims/instr.
  - scores: TWO fp8-DR matmuls; pass1 contracts {ck8*qe8, krot8*qrot8},
    pass2 the residual cross terms {ck8*dqe8, dck8*qe8}.  Half the PE
    cost of the f32r baseline.
  - es = exp(s) in bf16; PV is a bf16 matmul; Z = sum_k es via a bf16
    pairwise tree on DVE + one ones-matmul (frees PE).  ones = 1/DS so
    o1 = lat*DS directly.
  - out-proj: lat as 2-level fp8 (lat8 + dlat8), U as 2-level fp8
    (U8 + dU8, x64); three fp8-DR passes lat8@U8 + dlat8@U8 + lat8@dU8
    (the dlat@dU term is negligible) = 0.75x bf16 PE cost.

Pipeline: A down-proj + LOCAL transposes/rope of this rank's 512 tokens
(feat-major conversion happens before the gather: 4 tiles instead of
16) -> AG0 gathers {tok-major ckv bf16, feat-major [ck8|krot8|dck8]
fp8, feat-major c_q bf16} per token half -> DMA assembly -> B q_eff/
q_rot (bf16) -> C attention (paired [128,1024] exps, 2-bank PSUM
tiles) -> AG lat8/dlat8 fp8 -> E out-proj.  Mid-compute stores ride the
gpsimd queue (25ns dispatch vs 565-667ns on SP/Act).
"""
import sys
if '/opt/trn_rl_repo' not in sys.path:
    sys.path.insert(0, '/opt/trn_rl_repo')
import numpy as np
import ml_dtypes

D_MODEL = 5120
N_HEADS = 8
D_HEAD = 640
D_ROPE = 16
SPLIT = 624
D_KV = 128
BATCH, SEQ = 2, 2048
ROPE_SCALE = 40.0
SCALE = 1.0 / np.sqrt(np.float32(D_HEAD))
QS = 128.0          # q-side fp8 pre-scale (scores descaled in exp)
US = 64.0           # U fp8 pre-scale
DS = 256.0          # d' (lat residual) fp8 pre-scale
S128 = float(SCALE * QS)

N_CORES = 8
TP = 4
HL = 2
ESL = D_MODEL // TP          # 1280
DM_CH = D_MODEL // 128       # 40
NROT = N_HEADS * D_ROPE      # 128
NKRS = 64
NDOWN = 2 * D_KV + NROT + NKRS   # 448
TOKT = SEQ // 128            # 16
TOKC = SEQ // 512            # 4
LATF = N_HEADS * D_KV        # 1024
E_CH = [(0, 512), (512, 512), (1024, 256)]

_CACHE = {}
LAST_RESULTS = None


def _build_nc(with_ag=True):
    import concourse.bacc as bacc
    import concourse.mybir as mybir
    import concourse.tile as tile

    f32 = mybir.dt.float32
    f32r = mybir.dt.float32r
    bf16 = mybir.dt.bfloat16
    fp8 = mybir.dt.float8e4
    Exp = mybir.ActivationFunctionType.Exp
    Ident = mybir.ActivationFunctionType.Identity
    DR = mybir.MatmulPerfMode.DoubleRow
    Sub = mybir.AluOpType.subtract
    Mult = mybir.AluOpType.mult
    Add = mybir.AluOpType.add

    nc = bacc.Bacc("TRN2", target_bir_lowering=False, debug=False,
                   num_devices=N_CORES)

    # pre-tiled on host: col block d holds DRAM rows d*128..d*128+128
    hT8 = nc.dram_tensor("hT8", [128, DM_CH * (SEQ // TP)], fp8,
                         kind="ExternalInput").ap()
    dhT8 = nc.dram_tensor("dhT8", [128, DM_CH * (SEQ // TP)], fp8,
                          kind="ExternalInput").ap()
    Wd8 = nc.dram_tensor("Wd8", [128, DM_CH * NDOWN], fp8,
                         kind="ExternalInput").ap()
    dWd8 = nc.dram_tensor("dWd8", [128, DM_CH * NDOWN], fp8,
                          kind="ExternalInput").ap()
    down_bias = nc.dram_tensor("down_bias", [128, NDOWN], f32,
                               kind="ExternalInput").ap()
    Wm = nc.dram_tensor("Wm", [D_KV, HL * 128], bf16,
                        kind="ExternalInput").ap()
    mbias = nc.dram_tensor("mbias", [128, HL], f32, kind="ExternalInput").ap()
    Wqr = nc.dram_tensor("Wqr", [D_KV, HL * NROT], bf16,
                         kind="ExternalInput").ap()
    Wqrs = nc.dram_tensor("Wqrs", [D_KV, HL * 64], bf16,
                          kind="ExternalInput").ap()
    qrbias = nc.dram_tensor("qrbias", [128, HL], f32, kind="ExternalInput").ap()
    qrbias_sw = nc.dram_tensor("qrbias_sw", [64, HL], f32,
                               kind="ExternalInput").ap()
    costab = nc.dram_tensor("costab", [64, SEQ], bf16, kind="ExternalInput").ap()
    sinabs = nc.dram_tensor("sinabs", [64, SEQ], bf16, kind="ExternalInput").ap()
    costab_l = nc.dram_tensor("costab_l", [64, SEQ // TP], bf16,
                              kind="ExternalInput").ap()
    sinabs_l = nc.dram_tensor("sinabs_l", [64, SEQ // TP], bf16,
                              kind="ExternalInput").ap()
    ones_in = nc.dram_tensor("ones", [128, 1], bf16, kind="ExternalInput").ap()
    ident_in = nc.dram_tensor("ident", [128, 128], bf16,
                              kind="ExternalInput").ap()
    U8_in = nc.dram_tensor("U8", [128, (LATF // 128) * ESL], fp8,
                           kind="ExternalInput").ap()
    dU8_in = nc.dram_tensor("dU8", [128, (LATF // 128) * ESL], fp8,
                            kind="ExternalInput").ap()
    bo = nc.dram_tensor("bo", [128, ESL], f32, kind="ExternalInput").ap()
    out = nc.dram_tensor("out", [SEQ, ESL], f32, kind="ExternalOutput").ap()

    from contextlib import ExitStack
    with tile.TileContext(nc) as tc:
        with ExitStack() as _stk:
            dram = _stk.enter_context(
                tc.tile_pool(name="dram", bufs=1, space="DRAM"))
            keep = _stk.enter_context(tc.tile_pool(name="keep", bufs=1))
            _inner = ExitStack()
            cst = _inner.enter_context(tc.tile_pool(name="const", bufs=1))
            cTp = _inner.enter_context(tc.tile_pool(name="cT", bufs=1))
            dbias_t = cst.tile([128, NDOWN], f32, name="dbias_t")
            nc.gpsimd.dma_start(dbias_t[:], down_bias)
            mb_t = cst.tile([128, HL], f32, name="mb_t")
            nc.gpsimd.dma_start(mb_t[:], mbias)
            qrb_t = cst.tile([128, HL], f32, name="qrb_t")
            nc.gpsimd.dma_start(qrb_t[:], qrbias)
            ones_t = cst.tile([128, 1], bf16, name="ones_t")
            nc.gpsimd.dma_start(ones_t[:], ones_in)
            ident_t = cst.tile([128, 128], bf16, name="ident_t")
            nc.gpsimd.dma_start(ident_t[:], ident_in)
            wm_t = cst.tile([128, HL * 128], bf16, name="wm_t")
            nc.gpsimd.dma_start(wm_t[:], Wm)
            wqr_t = cst.tile([128, HL * NROT], bf16, name="wqr_t")
            nc.gpsimd.dma_start(wqr_t[:], Wqr)
            wqrs_t = cst.tile([128, HL * 64], bf16, name="wqrs_t")
            nc.gpsimd.dma_start(wqrs_t[:], Wqrs)
            qrbs_t = cst.tile([64, HL], f32, name="qrbs_t")
            nc.gpsimd.dma_start(qrbs_t[:], qrbias_sw)
            cs_t = cst.tile([64, SEQ], bf16, name="cs_t")
            sq_t = cst.tile([64, SEQ], bf16, name="sq_t")
            nc.gpsimd.dma_start(cs_t[:], costab)
            nc.gpsimd.dma_start(sq_t[:], sinabs)
            cs_l = cst.tile([64, SEQ // TP], bf16, name="cs_l")
            sq_l = cst.tile([64, SEQ // TP], bf16, name="sq_l")
            nc.gpsimd.dma_start(cs_l[:], costab_l)
            nc.gpsimd.dma_start(sq_l[:], sinabs_l)
            bo_t = keep.tile([128, ESL], f32, name="bo_t")
            nc.gpsimd.dma_start(bo_t[:], bo)

            # fp8 key-side pack: [ck8 | krot8 | dck8] along free dim
            ck3 = cTp.tile([128, 3 * SEQ], fp8, name="ck3")
            ck3v = ck3[:, :].rearrange("p (three n) -> p three n", three=3)
            # fp8 q-side pack per head: [qe8 | qrot8 | dqe8]
            qq3 = [cTp.tile([128, 3 * SEQ], fp8, name=f"qq3_{l}")
                   for l in range(HL)]
            qq3v = [t[:, :].rearrange("p (three n) -> p three n", three=3)
                    for t in qq3]
            c_qT_c = [cTp.tile([128, 512], bf16, name=f"c_qT{i}")
                      for i in range(TOKC)]
            # tok-major c_kv (PV lhsT / Vsum), one tile per global token tile
            a_ck = [cTp.tile([128, 128], bf16, name=f"ack{t}")
                    for t in range(TOKT)]

            ag_in = [dram.tile([2 * HL * 128, SEQ // 2], fp8, name=f"ag_in{i}")
                     for i in range(2)]
            ag_out = [dram.tile([2 * LATF, SEQ // 2], fp8, name=f"ag_out{i}")
                      for i in range(2)]
            # AG0 wire, per half of this rank's 512 tokens:
            #  w1 [256,128] bf16 tok-major c_kv; w2 [128, 3*256] fp8
            #  feat-major [ck8|krot8|dck8]; w3 [128,256] bf16 feat-major c_q
            w1_in = [dram.tile([256, 128], bf16, name=f"w1i{i}")
                     for i in range(2)]
            w2_in = [dram.tile([128, 3 * 256], fp8, name=f"w2i{i}")
                     for i in range(2)]
            w3_in = [dram.tile([128, 256], bf16, name=f"w3i{i}")
                     for i in range(2)]
            w1_out = [dram.tile([TP * 256, 128], bf16, name=f"w1o{i}")
                      for i in range(2)]
            w2_out = [dram.tile([TP * 128, 3 * 256], fp8, name=f"w2o{i}")
                      for i in range(2)]
            w3_out = [dram.tile([TP * 128, 256], bf16, name=f"w3o{i}")
                      for i in range(2)]

            # ---------- phase A: down-proj (fp8 3-pass) + local transposes --
            with tc.tile_pool(name="wdown", bufs=1) as wdp, \
                 tc.tile_pool(name="atm", bufs=4) as atp, \
                 tc.tile_pool(name="prT", bufs=6) as prT, \
                 tc.tile_pool(name="psA", bufs=4, space="PSUM") as psA, \
                 tc.tile_pool(name="psT", bufs=4, space="PSUM") as psT:
                wd_c, dwd_c, h_c, dh_c = [], [], [], []
                for ch in range(4):
                    w = wdp.tile([128, 10 * NDOWN], fp8, name=f"wd_c{ch}")
                    nc.sync.dma_start(
                        w[:], Wd8[:, ch * 10 * NDOWN:(ch + 1) * 10 * NDOWN])
                    wd_c.append(w)
                    h = wdp.tile([128, 10 * 512], fp8, name=f"h_c{ch}")
                    nc.scalar.dma_start(
                        h[:], hT8[:, ch * 10 * 512:(ch + 1) * 10 * 512])
                    h_c.append(h)
                for ch in range(4):
                    dw = wdp.tile([128, 10 * NDOWN], fp8, name=f"dwd_c{ch}")
                    nc.gpsimd.dma_start(
                        dw[:], dWd8[:, ch * 10 * NDOWN:(ch + 1) * 10 * NDOWN])
                    dwd_c.append(dw)
                    dh = wdp.tile([128, 10 * 512], fp8, name=f"dh_c{ch}")
                    nc.scalar.dma_start(
                        dh[:], dhT8[:, ch * 10 * 512:(ch + 1) * 10 * 512])
                    dh_c.append(dh)

                def a_pass(pst, tt, hs, ws, first, last):
                    # 20 DR steps contracting dm-chunk pairs (pairs stay
                    # within one 10-chunk tile: 5 pairs per tile)
                    for dp in range(DM_CH // 2):
                        ch, dd = divmod(2 * dp, 10)
                        hv = hs[ch][:, :].rearrange(
                            "p (k n) -> p k n", n=512)[
                                :, dd:dd + 2, tt * 128:(tt + 1) * 128]
                        wv = ws[ch][:, :].rearrange(
                            "p (k n) -> p k n", n=NDOWN)[:, dd:dd + 2, :]
                        nc.tensor.matmul(
                            pst[:], hv, wv,
                            start=(first and dp == 0),
                            stop=(last and dp == DM_CH // 2 - 1),
                            perf_mode=DR, skip_group_check=True)

                for tt in range(4):
                    pst = psA.tile([128, NDOWN], f32, tag="psA",
                                   name=f"psA_{tt}")
                    a_pass(pst, tt, h_c, wd_c, True, False)
                    a_pass(pst, tt, dh_c, wd_c, False, False)
                    a_pass(pst, tt, h_c, dwd_c, False, True)
                    a_tm = atp.tile([128, NDOWN], bf16, tag="atm",
                                    name=f"atm{tt}")
                    nc.vector.scalar_tensor_tensor(
                        a_tm[:], pst[:], 1.0 / 64.0, dbias_t[:], Mult, Add)
                    half, sl2 = divmod(tt, 2)
                    lsl = slice(sl2 * 128, (sl2 + 1) * 128)
                    nc.sync.dma_start(w1_in[half][lsl, :], a_tm[:, 0:128])
                    # local transposes: ckv / cq / krot / krs
                    p4 = psT.tile([128, 128], bf16, tag="t",
                                  name=f"pT_{tt}")
                    nc.tensor.transpose(p4[:], a_tm[:, 0:128], ident_t[:])
                    ck8l = prT.tile([128, 128], fp8, tag="ck8l",
                                    name=f"ck8l{tt}")
                    nc.scalar.activation(ck8l[:], p4[:], Ident)
                    nc.sync.dma_start(w2_in[half][:, lsl], ck8l[:])
                    dck = prT.tile([128, 128], fp8, tag="dck",
                                   name=f"dck{tt}")
                    nc.vector.tensor_tensor(dck[:], p4[:], ck8l[:], Sub)
                    nc.gpsimd.dma_start(
                        w2_in[half][:, 512 + sl2 * 128:512 + (sl2 + 1) * 128],
                        dck[:])
                    pq = psT.tile([128, 128], bf16, tag="t", name=f"pQ_{tt}")
                    nc.tensor.transpose(pq[:], a_tm[:, 128:256], ident_t[:])
                    cqb = prT.tile([128, 128], bf16, tag="cqb",
                                   name=f"cqb{tt}")
                    nc.vector.tensor_copy(cqb[:], pq[:])
                    nc.sync.dma_start(w3_in[half][:, lsl], cqb[:])
                    pk = psT.tile([128, 128], bf16, tag="t", name=f"pK_{tt}")
                    nc.tensor.transpose(pk[:], a_tm[:, 256:384], ident_t[:])
                    pks = psT.tile([64, 128], bf16, tag="t", name=f"pS_{tt}")
                    nc.tensor.transpose(pks[:], a_tm[:, 384:448],
                                        ident_t[:])
                    # krot: passive rows straight out; active rows roped
                    kr8 = prT.tile([128, 128], fp8, tag="kr8",
                                   name=f"kr8{tt}")
                    nc.scalar.activation(kr8[64:128, :], pk[64:128, :], Ident)
                    krA = prT.tile([64, 128], bf16, tag="krA",
                                   name=f"krA{tt}")
                    tsl = slice(tt * 128, (tt + 1) * 128)
                    nc.vector.tensor_mul(krA[:], pk[0:64, :], cs_l[:, tsl])
                    krs = prT.tile([64, 128], bf16, tag="krs",
                                   name=f"krs{tt}")
                    nc.vector.tensor_mul(krs[:], pks[:], sq_l[:, tsl])
                    nc.vector.tensor_add(kr8[0:64, :], krA[:], krs[:])
                    nc.gpsimd.dma_start(
                        w2_in[half][:, 256 + sl2 * 128:256 + (sl2 + 1) * 128],
                        kr8[:])
                    if tt % 2 == 1:
                        if with_ag:
                            for wi, wo in [(w1_in, w1_out), (w2_in, w2_out),
                                           (w3_in, w3_out)]:
                                nc.gpsimd.collective_compute(
                                    "AllGather", mybir.AluOpType.bypass,
                                    replica_groups=[[0, 1, 2, 3],
                                                    [4, 5, 6, 7]],
                                    ins=[wi[half].opt()],
                                    outs=[wo[half].opt()])
                        else:
                            nc.gpsimd.dma_start(w1_out[half][0:256, :],
                                                w1_in[half][:])
                            nc.gpsimd.dma_start(w2_out[half][0:128, :],
                                                w2_in[half][:])
                            nc.gpsimd.dma_start(w3_out[half][0:128, :],
                                                w3_in[half][:])

            with tc.tile_pool(name="expp", bufs=11) as expp, \
                 tc.tile_pool(name="zpp", bufs=10) as zpp, \
                 tc.tile_pool(name="ev", bufs=6) as evp, \
                 tc.tile_pool(name="ps_s", bufs=2, space="PSUM") as ps_s, \
                 tc.tile_pool(name="ps_pv", bufs=2, space="PSUM") as ps_pv, \
                 tc.tile_pool(name="ps_z", bufs=1, space="PSUM") as ps_z:
                # ---------- DMA assembly of gathered feat-major tiles ------
                # w1_out rows: rank r block of 256 tok (per half) -> a_ck
                # w2_out rows: rank r block of 128 feats, col groups
                # [ck8|krot8|dck8] of 256 tok -> ck3; w3_out -> c_qT_c[r]
                for half in range(2):
                    # c_qT first (B's critical path), then ck3, then a_ck
                    for r in range(TP):
                        nc.scalar.dma_start(
                            c_qT_c[r][:, half * 256:(half + 1) * 256],
                            w3_out[half][r * 128:(r + 1) * 128, :])
                    for r in range(TP):
                        # one DMA per (half, rank): contiguous [128,768]
                        # source, 3-dim dest AP striding the three ck3
                        # k-tile segments (one dispatch instead of three)
                        dst = ck3v[:, :, r * 512 + half * 256:
                                   r * 512 + half * 256 + 256]
                        q = nc.sync if r % 2 == 0 else nc.scalar
                        q.dma_start(
                            dst,
                            w2_out[half][r * 128:(r + 1) * 128, :]
                            .rearrange("p (three n) -> p three n", three=3))
                    for r in range(TP):
                        for s2 in range(2):
                            t = 4 * r + half * 2 + s2
                            q = nc.sync if t % 2 == 0 else nc.scalar
                            q.dma_start(
                                a_ck[t][:],
                                w1_out[half][r * 256 + s2 * 128:
                                             r * 256 + (s2 + 1) * 128, :])

                # ---------- phase B: q_eff, q_rot + RoPE ----------
                for hl in range(HL):
                    for tc4 in range(TOKC):
                        sl = slice(tc4 * 512, (tc4 + 1) * 512)
                        sl1 = slice(SEQ + tc4 * 512, SEQ + (tc4 + 1) * 512)
                        sl2 = slice(2 * SEQ + tc4 * 512, 2 * SEQ + (tc4 + 1) * 512)
                        ps = ps_s.tile([128, 512], f32, tag="s",
                                       name=f"psqe{hl}_{tc4}")
                        nc.tensor.matmul(
                            ps[:], wm_t[:, hl * 128:(hl + 1) * 128],
                            c_qT_c[tc4][:], start=True, stop=True,
                            skip_group_check=True)
                        nc.scalar.activation(qq3[hl][:, sl], ps[:], Ident,
                                             bias=mb_t[:, hl:hl + 1],
                                             scale=S128)
                        # residual: dqe8 = ps*S128 - qe8
                        nc.vector.scalar_tensor_tensor(
                            qq3[hl][:, sl2], ps[:], S128, qq3[hl][:, sl],
                            Mult, Sub)
                        psr = ps_z.tile([128, 512], f32, tag="z",
                                        name=f"psqr{hl}_{tc4}")
                        nc.tensor.matmul(
                            psr[:], wqr_t[:, hl * NROT:(hl + 1) * NROT],
                            c_qT_c[tc4][:], start=True, stop=True,
                            skip_group_check=True)
                        nc.scalar.activation(
                            qq3[hl][64:128, sl1], psr[64:128, :], Ident,
                            bias=qrb_t[64:128, hl:hl + 1], scale=S128)
                        qrA = expp.tile([64, 512], bf16, tag="rope512",
                                        name=f"qrA{hl}_{tc4}")
                        nc.vector.tensor_scalar(
                            qrA[:], psr[0:64, :], S128,
                            qrb_t[0:64, hl:hl + 1], Mult, Add)
                        psw = ps_z.tile([64, 512], f32, tag="z",
                                        name=f"psw{hl}_{tc4}")
                        nc.tensor.matmul(
                            psw[:], wqrs_t[:, hl * 64:(hl + 1) * 64],
                            c_qT_c[tc4][:], start=True, stop=True,
                            skip_group_check=True)
                        tmq = expp.tile([64, 512], bf16, tag="rope512",
                                        name=f"tmq{hl}_{tc4}")
                        nc.scalar.activation(tmq[:], psw[:], Ident,
                                             bias=qrbs_t[:, hl:hl + 1],
                                             scale=S128)
                        nc.vector.tensor_mul(tmq[:], tmq[:], sq_t[0:64, sl])
                        nc.vector.tensor_mul(qrA[:], qrA[:], cs_t[0:64, sl])
                        nc.vector.tensor_add(qq3[hl][0:64, sl1],
                                             qrA[:], tmq[:])

                # ---------- phase C ----------
                for qc in range(TOKC):
                    qsl = slice(qc * 512, (qc + 1) * 512)
                    for hl in range(HL):
                        pv = ps_pv.tile([128, 512], f32, tag="pv",
                                        name=f"pv{hl}_{qc}")
                        zp = []  # bf16 partial-sum tree tiles
                        for kp in range(TOKT // 2):
                            psS = ps_s.tile([128, 1024], f32, tag="s",
                                            name=f"psS{hl}_{qc}_{kp}")
                            esp = expp.tile([128, 1024], bf16, tag="expS",
                                            name=f"es{hl}_{qc}_{kp}")
                            for s2 in range(2):
                                kt = 2 * kp + s2
                                ksl = slice(kt * 128, (kt + 1) * 128)
                                nc.tensor.matmul(
                                    psS[:, s2 * 512:(s2 + 1) * 512],
                                    ck3v[:, 0:2, ksl], qq3v[hl][:, 0:2, qsl],
                                    start=True, stop=False, perf_mode=DR,
                                    skip_group_check=True)
                                nc.tensor.matmul(
                                    psS[:, s2 * 512:(s2 + 1) * 512],
                                    ck3v[:, 0::2, ksl],
                                    qq3v[hl][:, 2::-2, qsl],
                                    start=False, stop=True, perf_mode=DR,
                                    skip_group_check=True)
                            nc.scalar.activation(esp[:], psS[:], Exp,
                                                 scale=float(1.0 / QS))
                            for s2 in range(2):
                                kt = 2 * kp + s2
                                nc.tensor.matmul(
                                    pv[:], a_ck[kt][:],
                                    esp[:, s2 * 512:(s2 + 1) * 512],
                                    start=(kt == 0), stop=(kt == TOKT - 1),
                                    skip_group_check=True)
                            z1 = zpp.tile([128, 512], bf16, tag="zp",
                                          name=f"z1_{hl}_{qc}_{kp}")
                            nc.vector.tensor_add(z1[:], esp[:, 0:512],
                                                 esp[:, 512:1024])
                            zp.append(z1)
                        # DVE pairwise tree over the 8 partials
                        lvl = 1
                        while len(zp) > 1:
                            nxt = []
                            for j in range(0, len(zp), 2):
                                zn = zpp.tile([128, 512], bf16, tag="zp",
                                              name=f"zt{lvl}_{hl}_{qc}_{j}")
                                nc.vector.tensor_add(zn[:], zp[j][:],
                                                     zp[j + 1][:])
                                nxt.append(zn)
                            zp = nxt
                            lvl += 1
                        zt = ps_z.tile([1, 512], f32, tag="z",
                                       name=f"z{hl}_{qc}")
                        nc.tensor.matmul(zt[:], ones_t[:], zp[0][:],
                                         start=True, stop=True,
                                         skip_group_check=True)
                        rz = evp.tile([1, 512], f32, tag="rz",
                                      name=f"rz{hl}_{qc}")
                        nc.vector.reciprocal(rz[:], zt[:])
                        rzb = evp.tile([128, 512], f32, tag="rzb",
                                       name=f"rzb{hl}_{qc}")
                        nc.gpsimd.partition_broadcast(rzb[:], rz[:])
                        # ones_t holds 1/DS so rzb = DS/Z and o1 = lat*DS
                        o1 = evp.tile([128, 512], f32, tag="o1",
                                      name=f"o1_{hl}_{qc}")
                        nc.vector.tensor_mul(o1[:], pv[:], rzb[:])
                        half, qq = divmod(qc, 2)
                        l8 = evp.tile([128, 512], fp8, tag="d8",
                                      name=f"l8_{hl}_{qc}")
                        nc.vector.tensor_copy(l8[:], o1[:])
                        dl8 = evp.tile([128, 512], fp8, tag="dl8",
                                       name=f"dl8_{hl}_{qc}")
                        nc.vector.tensor_tensor(dl8[:], o1[:], l8[:], Sub)
                        nc.gpsimd.dma_start(
                            ag_in[half][hl * 128:(hl + 1) * 128,
                                        qq * 512:(qq + 1) * 512], l8[:])
                        nc.gpsimd.dma_start(
                            ag_in[half][256 + hl * 128:256 + (hl + 1) * 128,
                                        qq * 512:(qq + 1) * 512], dl8[:])
                    if qc % 2 == 1:
                        half = qc // 2
                        if with_ag:
                            nc.gpsimd.collective_compute(
                                "AllGather", mybir.AluOpType.bypass,
                                replica_groups=[[0, 1, 2, 3], [4, 5, 6, 7]],
                                ins=[ag_in[half].opt()],
                                outs=[ag_out[half].opt()])
                        else:
                            nc.gpsimd.dma_start(
                                ag_out[half][0:2 * HL * 128, :],
                                ag_in[half][:])

            # ---------- phase E: output projection, per token half ----------
            _inner.close()
            with tc.tile_pool(name="slab", bufs=2) as slp, \
                 tc.tile_pool(name="uw", bufs=1) as uwp, \
                 tc.tile_pool(name="oev", bufs=2) as oev, \
                 tc.tile_pool(name="psE", bufs=1, space="PSUM") as psE:
                L_CH = LATF // 128  # 8
                u8 = uwp.tile([128, L_CH * ESL], fp8, name="u8")
                nc.sync.dma_start(u8[:], U8_in)
                u8v = u8[:, :].rearrange("p (k e) -> p k e", k=L_CH)
                du8 = uwp.tile([128, L_CH * ESL], fp8, name="du8")
                nc.sync.dma_start(du8[:], dU8_in)
                du8v = du8[:, :].rearrange("p (k e) -> p k e", k=L_CH)
                EVS = float(1.0 / (US * DS))
                for th in range(2):
                    # lat8 rows at rank*512, dlat8 rows at rank*512+256
                    slab = slp.tile([128, L_CH * 1024], fp8, tag="slab",
                                    name=f"slab{th}")
                    slabv = slab[:, :].rearrange("p (k t) -> p k t", k=L_CH)
                    slad = slp.tile([128, L_CH * 1024], fp8, tag="slad",
                                    name=f"slad{th}")
                    sladv = slad[:, :].rearrange("p (k t) -> p k t", k=L_CH)
                    for r in range(L_CH // 2):
                        # paired lat-chunks (hl0+hl1 of rank r are 256
                        # contiguous ag_out rows): one dispatch for two
                        # chunks via matched 3-dim APs
                        src = ag_out[th][r * 512:r * 512 + 256, :]\
                            .rearrange("(two p) c -> p two c", two=2)
                        dst = slab[:, 2 * r * 1024:(2 * r + 2) * 1024]\
                            .rearrange("p (two c) -> p two c", two=2)
                        nc.scalar.dma_start(dst, src)
                        srd = ag_out[th][r * 512 + 256:r * 512 + 512, :]\
                            .rearrange("(two p) c -> p two c", two=2)
                        dsd = slad[:, 2 * r * 1024:(2 * r + 2) * 1024]\
                            .rearrange("p (two c) -> p two c", two=2)
                        nc.scalar.dma_start(dsd, srd)
                    for pr in range(4):
                        oe = [oev.tile([128, ESL], f32, tag=f"oe{s}",
                                       name=f"oe_{th}_{pr}_{s}")
                              for s in range(2)]
                        for eoff, ew in E_CH:
                            pse = [psE.tile([128, 512], f32, tag=f"e{s}_{eoff}",
                                            name=f"psE_{th}_{pr}_{eoff}_{s}")
                                   for s in range(2)]
                            for s in range(2):
                                t8 = pr * 2 + s
                                tok = slice(t8 * 128, (t8 + 1) * 128)
                                # (lat8+dlat8)@(U8+dU8) minus dlat@dU:
                                # pass1 lat8*U8, pass2 dlat8*U8, pass3
                                # lat8*dU8 -- all into one PSUM group
                                for pj, (sv, uv) in enumerate(
                                        [(slabv, u8v), (sladv, u8v),
                                         (slabv, du8v)]):
                                    for kj in range(L_CH // 2):
                                        nc.tensor.matmul(
                                            pse[s][:, :ew],
                                            sv[:, 2 * kj:2 * kj + 2, tok],
                                            uv[:, 2 * kj:2 * kj + 2,
                                               eoff:eoff + ew],
                                            start=(pj == 0 and kj == 0),
                                            stop=(pj == 2 and
                                                  kj == L_CH // 2 - 1),
                                            perf_mode=DR,
                                            skip_group_check=True)
                            for s in range(2):
                                nc.vector.scalar_tensor_tensor(
                                    oe[s][:, eoff:eoff + ew], pse[s][:, :ew],
                                    EVS, bo_t[:, eoff:eoff + ew], Mult, Add)
                        for s in range(2):
                            tok = th * 8 + pr * 2 + s
                            nc.sync.dma_start(
                                out[tok * 128:(tok + 1) * 128, :], oe[s][:])

    nc.compile()
    return nc


def _rope_tables():
    """[64, SEQ] tables for the packed active-dims layout."""
    inv_freq = (1.0 / (10000.0 ** (np.arange(0, D_ROPE // 2, 2, dtype=np.float32)
                                   / (D_ROPE // 2)))).astype(np.float32)
    t = np.arange(SEQ, dtype=np.float32) / np.float32(ROPE_SCALE)
    freqs = t[:, None] * inv_freq[None, :]          # [SEQ, 4]
    cos = np.cos(freqs).astype(np.float32).T        # [4, SEQ]
    sin = np.sin(freqs).astype(np.float32).T
    costab = np.empty((64, SEQ), np.float32)
    sintab = np.empty((64, SEQ), np.float32)
    sinabs = np.empty((64, SEQ), np.float32)
    for p in range(8):
        j = p % 4
        costab[np.arange(N_HEADS) * 8 + p] = cos[j]
        sintab[np.arange(N_HEADS) * 8 + p] = -sin[j] if p < 4 else sin[j]
        sinabs[np.arange(N_HEADS) * 8 + p] = sin[j]
    return costab, sintab, sinabs


ROT_PERM = np.concatenate(
    [(np.arange(N_HEADS) * D_ROPE)[:, None] + np.arange(8)[None, :],
     (np.arange(N_HEADS) * D_ROPE)[:, None] + 8 + np.arange(8)[None, :]],
    axis=0).reshape(-1)


_FOLD_CACHE = {}


def _fold(W_uq, b_uq, W_uk, W_uv, b_uv, W_o, b_o):
    key = (W_uq.ctypes.data, W_uk.ctypes.data, W_uv.ctypes.data,
           W_o.ctypes.data)
    hit = _FOLD_CACHE.get(key)
    if hit is not None:
        return hit
    M = np.empty((N_HEADS, D_KV, 128), np.float32)
    m0 = np.empty((N_HEADS, 128), np.float32)
    U = np.empty((LATF, D_MODEL), np.float32)
    bo_eff = b_o.astype(np.float32).copy()
    for h in range(N_HEADS):
        Wuq_h = W_uq[:, h * SPLIT:(h + 1) * SPLIT]
        Wuk_h = W_uk[:, h * SPLIT:(h + 1) * SPLIT]
        M[h] = Wuq_h @ Wuk_h.T
        m0[h] = b_uq[h * SPLIT:(h + 1) * SPLIT] @ Wuk_h.T
        Wuv_h = W_uv[:, h * D_HEAD:(h + 1) * D_HEAD]
        Wo_h = W_o[h * D_HEAD:(h + 1) * D_HEAD, :]
        U[h * D_KV:(h + 1) * D_KV] = Wuv_h @ Wo_h
        bo_eff += b_uv[h * D_HEAD:(h + 1) * D_HEAD] @ Wo_h
    U8 = (U * US).astype(ml_dtypes.float8_e4m3)
    dU8 = (U * US - U8.astype(np.float32)).astype(ml_dtypes.float8_e4m3)
    res = (M, m0, U8, dU8, bo_eff)
    _FOLD_CACHE[key] = res
    return res


def _shard(inp):
    f32 = np.float32
    h = np.asarray(inp['h'], f32)
    W_dkv = np.asarray(inp['W_dkv'], f32); b_dkv = np.asarray(inp['b_dkv'], f32)
    W_dq = np.asarray(inp['W_dq'], f32); b_dq = np.asarray(inp['b_dq'], f32)
    W_uk = np.asarray(inp['W_uk'], f32); b_uk = np.asarray(inp['b_uk'], f32)
    W_uv = np.asarray(inp['W_uv'], f32); b_uv = np.asarray(inp['b_uv'], f32)
    W_uq = np.asarray(inp['W_uq'], f32); b_uq = np.asarray(inp['b_uq'], f32)
    W_qr = np.asarray(inp['W_qr'], f32); b_qr = np.asarray(inp['b_qr'], f32)
    W_kr = np.asarray(inp['W_kr'], f32); b_kr = np.asarray(inp['b_kr'], f32)
    W_o = np.asarray(inp['W_o'], f32); b_o = np.asarray(inp['b_o'], f32)

    M, m0, U8, dU8, bo_eff = _fold(W_uq, b_uq, W_uk, W_uv, b_uv, W_o, b_o)
    costab, sintab, sinabs = _rope_tables()
    bf = ml_dtypes.bfloat16
    hTs = [np.ascontiguousarray(h[b].T) for b in range(BATCH)]
    ident = np.eye(128, dtype=f32).astype(bf)
    ones = np.full((128, 1), 1.0 / DS, f32).astype(bf)

    in_maps = []
    def pretile(a, pr=128):
        n = a.shape[0] // pr
        return np.ascontiguousarray(
            a.reshape(n, pr, a.shape[1]).transpose(1, 0, 2).reshape(pr, -1))

    f8 = ml_dtypes.float8_e4m3
    Wkr_sw = np.zeros((D_MODEL, NKRS), f32)
    bkr_sw = np.zeros(NKRS, f32)
    for hh in range(N_HEADS):
        for p in range(8):
            sgn = -1.0 if p < 4 else 1.0
            src = hh * D_ROPE + (p + 4) % 8
            Wkr_sw[:, hh * 8 + p] = sgn * W_kr[:, src]
            bkr_sw[hh * 8 + p] = sgn * b_kr[src]
    Wcat = np.concatenate(
        [W_dkv, W_dq, W_kr[:, ROT_PERM], Wkr_sw], axis=1) * 64.0
    Wd8_np = Wcat.astype(f8)
    dWd8_np = (Wcat - Wd8_np.astype(f32)).astype(f8)
    Wd8_u = pretile(Wd8_np)
    dWd8_u = pretile(dWd8_np)
    db_row = np.concatenate([b_dkv, b_dq, b_kr[ROT_PERM], bkr_sw])
    db_u = np.ascontiguousarray(np.tile(db_row[None, :], (128, 1)), f32)

    for c in range(N_CORES):
        b, g = divmod(c, TP)
        heads = [2 * g, 2 * g + 1]
        Wm_c = np.concatenate([M[hh] for hh in heads], axis=1)
        mb = np.stack([m0[hh] * SCALE * QS for hh in heads], axis=1)
        Wqr_c = np.zeros((D_KV, HL * NROT), f32)
        qrb = np.zeros((128, HL), f32)
        Wqrs_c = np.zeros((D_KV, HL * 64), f32)
        qrbs = np.zeros((64, HL), f32)
        inv_perm = np.argsort(ROT_PERM)
        for hl, hh in enumerate(heads):
            for j in range(D_ROPE):
                r = inv_perm[hh * D_ROPE + j]
                Wqr_c[:, hl * NROT + r] = W_qr[:, hh * D_ROPE + j]
                qrb[r, hl] = b_qr[hh * D_ROPE + j] * SCALE * QS
            for p in range(8):
                sgn = -1.0 if p < 4 else 1.0
                src = hh * D_ROPE + (p + 4) % 8
                Wqrs_c[:, hl * 64 + hh * 8 + p] = sgn * W_qr[:, src]
                qrbs[hh * 8 + p, hl] = sgn * b_qr[src] * SCALE * QS
        esl = slice(g * ESL, (g + 1) * ESL)
        h_loc = np.ascontiguousarray(
            hTs[b][:, g * (SEQ // TP):(g + 1) * (SEQ // TP)])
        h8_loc = h_loc.astype(f8)
        dh8_loc = (h_loc - h8_loc.astype(f32)).astype(f8)
        tsl = slice(g * (SEQ // TP), (g + 1) * (SEQ // TP))
        in_maps.append({
            "hT8": pretile(h8_loc),
            "dhT8": pretile(dh8_loc),
            "Wd8": Wd8_u,
            "dWd8": dWd8_u,
            "down_bias": db_u,
            "Wm": np.ascontiguousarray(Wm_c.astype(bf)),
            "mbias": np.ascontiguousarray(mb, f32),
            "Wqr": Wqr_c.astype(bf),
            "Wqrs": Wqrs_c.astype(bf),
            "qrbias": qrb,
            "qrbias_sw": qrbs,
            "costab": costab.astype(bf),
            "sinabs": sinabs.astype(bf),
            "costab_l": np.ascontiguousarray(costab[:, tsl].astype(bf)),
            "sinabs_l": np.ascontiguousarray(sinabs[:, tsl].astype(bf)),
            "ones": ones,
            "ident": ident,
            "U8": pretile(np.ascontiguousarray(U8[:, esl])),
            "dU8": pretile(np.ascontiguousarray(dU8[:, esl])),
            "bo": np.ascontiguousarray(
                np.tile(bo_eff[esl][None, :], (128, 1)), f32),
        })
    return in_maps


def kernel(**inputs):
    global LAST_RESULTS
    from concourse import bass_utils
    if 'nc' not in _CACHE:
        _CACHE['nc'] = _build_nc()
    nc = _CACHE['nc']
    in_maps = _shard(inputs)
    res = bass_utils.run_bass_kernel_spmd(nc, in_maps,
                                          core_ids=list(range(N_CORES)))
    LAST_RESULTS = res
    out = np.empty((BATCH, SEQ, D_MODEL), np.float32)
    for c in range(N_CORES):
        b, g = divmod(c, TP)
        out[b, :, g * ESL:(g + 1) * ESL] = res.results[c]["out"]
    return out
